# revision 21
# baseline (speedup 1.0000x reference)
"""CropAndResize (tf.image.crop_and_resize semantics) on 8 Trainium2 cores.

Strategy
--------
Data-parallel over the 32 boxes/images: each core processes 4 "slots"
(images sorted by needed column-span, dealt round-robin).  The program is
one SPMD NEFF, but the body is 8 per-core branches on partition_id, each
fully specialized to that core's boxes: exact column windows for the
gathers, exact chunk counts, and column-matmuls trimmed to the j-range
each x-chunk actually feeds.  SBUF tiles are tagged per-slot (not
per-core) so the allocator sizes them once at the slot maximum.

Per image, on-device:
  1. Four indirect DMAs gather, per output row i, input rows top_i/bot_i
     restricted to the column window -> TB [128p, {g0t,g0b,g1t,g1b}, S*4]
     (224 output rows = two partition groups g: i = p + 128g).
  2. Row lerp in place: R_g = T_g*wt + B_g*wb (ScalarE scaled copies +
     VectorE add; per-partition scales).
  3. Column interp on TensorE: per channel c and 128-wide x-chunk,
     transpose R_g[:, chunk*4+c :: 4] (PE transpose -> PSUM -> copy to
     SBUF), then matmul against the host-built column-weight matrix
     Wx[x, j] over just the j-columns that chunk feeds, accumulating in
     PSUM via per-element has_written (start=True only on the first MM).
  4. Copy [i, j] PSUM results into the channel-interleaved output tile,
     DMA out.

All indices/weights come from the host (32*224 scalars) with float32 ops
exactly mirroring the reference, so validity masks and floor() match
bit-for-bit.
"""

import numpy as np

H = 1024
W = 1024
C = 4
CROP = 224
B = 32
NCORES = 8
SLOTS = B // NCORES  # 4
G = 2
ROW_ELEMS = W * C


# ----------------------------------------------------------------------------
# Host-side planning (exact float32 mirror of the reference index math)
# ----------------------------------------------------------------------------

def _axis_plan(lo, hi, n_in):
    grid = np.arange(CROP, dtype=np.float32) / np.float32(CROP - 1)
    pos = (lo + grid * (hi - lo)) * np.float32(n_in - 1)
    valid = (pos >= 0) & (pos <= n_in - 1)
    low_f = np.floor(pos)
    lerp = pos - low_f
    t = np.clip(low_f.astype(np.int32), 0, n_in - 1)
    b = np.clip(t + 1, 0, n_in - 1)
    wt = np.where(valid, np.float32(1.0) - lerp, np.float32(0.0)).astype(np.float32)
    wb = np.where(valid, lerp, np.float32(0.0)).astype(np.float32)
    return t, b, wt, wb


def _plan_image(box):
    y1, x1, y2, x2 = (np.float32(box[0]), np.float32(box[1]),
                      np.float32(box[2]), np.float32(box[3]))
    ty, by, wty, wby = _axis_plan(y1, y2, H)
    tx, bx, wtx, wbx = _axis_plan(x1, x2, W)
    xlo = int(tx.min())
    span = int(bx.max()) - xlo + 1
    span = min(max(32, (span + 3) // 4 * 4), W)  # small alignment pad
    xlo = min(xlo, W - span)
    lrel = tx - xlo
    rrel = bx - xlo
    K = (span + 127) // 128
    jranges = []
    for k in range(K):
        sel = (np.minimum(lrel, rrel) < 128 * (k + 1)) & \
              (np.maximum(lrel, rrel) >= 128 * k)
        js = np.nonzero(sel)[0]
        if len(js) == 0:
            jranges.append(None)
        else:
            jranges.append((int(js[0]), int(js[-1]) + 1))
    return dict(ty=ty, by=by, wty=wty, wby=wby, lrel=lrel, rrel=rrel,
                wtx=wtx, wbx=wbx, xlo=xlo, span=span, K=K, jranges=jranges)


def _make_plans(boxes):
    plans = [_plan_image(boxes[b]) for b in range(B)]
    order = sorted(range(B), key=lambda b: -plans[b]["span"])
    assignment = [[-1] * SLOTS for _ in range(NCORES)]
    for s in range(SLOTS):
        grp = order[s * NCORES:(s + 1) * NCORES]
        for c in range(NCORES):
            assignment[c][s] = grp[c]
    kmax = [max(plans[assignment[c][s]]["K"] for c in range(NCORES))
            for s in range(SLOTS)]
    return plans, assignment, kmax


def _program_key(plans, assignment):
    # everything the generated program depends on
    key = []
    for c in range(NCORES):
        for s in range(SLOTS):
            p = plans[assignment[c][s]]
            key.append((p["span"], tuple(p["jranges"])))
    return tuple(key)


def _build_host_inputs(x, boxes, plans, assignment, kmax):
    ktot = sum(kmax)
    in_maps = []
    for c in range(NCORES):
        imgs = [assignment[c][s] for s in range(SLOTS)]
        ximg = np.ascontiguousarray(x[imgs]).reshape(-1)
        rix = np.zeros((SLOTS, 128, 4), dtype=np.int32)
        ylw = np.zeros((SLOTS, 128, 4), dtype=np.float32)
        wxm = np.zeros((ktot * 128, 256), dtype=np.float32)
        koff = 0
        for s in range(SLOTS):
            p = plans[imgs[s]]
            base = s * H * ROW_ELEMS + p["xlo"] * C
            for g in range(G):
                i = np.arange(128) + 128 * g
                i = np.minimum(i, CROP - 1)
                pad = (np.arange(128) + 128 * g) >= CROP
                rix[s, :, 2 * g + 0] = base + p["ty"][i] * ROW_ELEMS
                rix[s, :, 2 * g + 1] = base + p["by"][i] * ROW_ELEMS
                ylw[s, :, 2 * g + 0] = np.where(pad, 0.0, p["wty"][i])
                ylw[s, :, 2 * g + 1] = np.where(pad, 0.0, p["wby"][i])
            wx = np.zeros((p["K"] * 128, 256), dtype=np.float32)
            j = np.arange(CROP)
            np.add.at(wx, (p["lrel"], j), p["wtx"])
            np.add.at(wx, (p["rrel"], j), p["wbx"])
            wxm[koff * 128:koff * 128 + p["K"] * 128] = wx
            koff += kmax[s]
        in_maps.append({"ximg": ximg, "rix": rix, "ylw": ylw, "wxm": wxm})
    return in_maps


# ----------------------------------------------------------------------------
# Device program
# ----------------------------------------------------------------------------

_PROGRAM_CACHE = {}


def _build_program(plans, assignment, kmax):
    key = _program_key(plans, assignment)
    if key in _PROGRAM_CACHE:
        return _PROGRAM_CACHE[key]

    import concourse.bass as bass
    import concourse.tile as tile
    from concourse import bacc, mybir
    from concourse.masks import make_identity

    f32 = mybir.dt.float32
    nc = bacc.Bacc("TRN2", target_bir_lowering=False, debug=False,
                   enable_asserts=False)

    ktot = sum(kmax)
    tot = SLOTS * H * ROW_ELEMS
    ximg = nc.dram_tensor("ximg", [tot], f32, kind="ExternalInput").ap()
    rix = nc.dram_tensor("rix", [SLOTS, 128, 4], mybir.dt.int32,
                         kind="ExternalInput").ap()
    ylw = nc.dram_tensor("ylw", [SLOTS, 128, 4], f32, kind="ExternalInput").ap()
    wxm = nc.dram_tensor("wxm", [ktot * 128, 256], f32,
                         kind="ExternalInput").ap()
    outp = nc.dram_tensor("out", [SLOTS, CROP, CROP * C], f32,
                          kind="ExternalOutput").ap()

    with tile.TileContext(nc) as tc:
        with (
            tc.tile_pool(name="meta", bufs=1) as meta_pool,
            tc.tile_pool(name="tb", bufs=1) as tb_pool,
            tc.tile_pool(name="rt", bufs=3) as rt_pool,
            tc.tile_pool(name="small", bufs=2) as small_pool,
            tc.tile_pool(name="pst", bufs=3, space="PSUM") as pst_pool,
            tc.tile_pool(name="pso", bufs=4, space="PSUM") as pso_pool,
        ):
            ident = meta_pool.tile([128, 128], f32, tag="ident")
            make_identity(nc, ident[:])

            # Uniform-shape metadata loads (outside the branches).
            metas = []
            for s in range(SLOTS):
                rix_sb = meta_pool.tile([128, 4], mybir.dt.int32, tag=f"rix{s}")
                nc.sync.dma_start(out=rix_sb[:], in_=rix[s])
                ylw_sb = meta_pool.tile([128, 4], f32, tag=f"ylw{s}")
                nc.sync.dma_start(out=ylw_sb[:], in_=ylw[s])
                metas.append((rix_sb, ylw_sb))

            pid = nc.partition_id()

            for core in range(NCORES):
                with tc.If(pid == core):
                    _emit_core(nc, tc, bass, mybir, f32,
                               [plans[assignment[core][s]] for s in range(SLOTS)],
                               kmax, ximg, wxm, outp, metas, ident,
                               tb_pool, rt_pool, small_pool, meta_pool,
                               pst_pool, pso_pool)

    nc.compile()
    _PROGRAM_CACHE[key] = nc
    return nc


def _emit_core(nc, tc, bass, mybir, f32, cplans, kmax, ximg, wxm, outp,
               metas, ident, tb_pool, rt_pool, small_pool, meta_pool,
               pst_pool, pso_pool):
    # per-core wx loads (per-core K counts)
    wxs = []
    koff = 0
    for s in range(SLOTS):
        K = cplans[s]["K"]
        wx_sb = meta_pool.tile([128, kmax[s], 256], f32, tag=f"wx{s}")
        nc.sync.dma_start(
            out=wx_sb[:, :K],
            in_=wxm[koff * 128:koff * 128 + K * 128].rearrange(
                "(k p) j -> p k j", p=128))
        koff += kmax[s]
        wxs.append(wx_sb)

    # all gathers first, smallest slot first (SWDGE completion-lane reuse
    # blocks the in-order Pool stream on transfer completion)
    order = sorted(range(SLOTS), key=lambda s: cplans[s]["span"])
    tbs = {}
    for s in order:
        F = cplans[s]["span"] * C
        rix_sb = metas[s][0]
        TB = tb_pool.tile([128, 4, F], f32, tag=f"TB{s}")
        for j in range(4):
            nc.gpsimd.indirect_dma_start(
                out=TB[:, j],
                out_offset=None,
                in_=ximg.rearrange("(n o) -> n o", o=1),
                in_offset=bass.IndirectOffsetOnAxis(
                    ap=rix_sb[:, j:j + 1], axis=0),
            )
        tbs[s] = TB

    for s in order:
        p = cplans[s]
        S, K = p["span"], p["K"]
        rix_sb, ylw_sb = metas[s]
        wx_sb = wxs[s]
        TB = tbs[s]

        # row lerp in place: TB[:,2g] = T_g*wt + B_g*wb
        for g in range(G):
            nc.scalar.mul(TB[:, 2 * g], TB[:, 2 * g],
                          ylw_sb[:, 2 * g:2 * g + 1])
            nc.vector.tensor_scalar_mul(
                TB[:, 2 * g + 1], TB[:, 2 * g + 1],
                ylw_sb[:, 2 * g + 1:2 * g + 2])
            nc.vector.tensor_add(TB[:, 2 * g], TB[:, 2 * g],
                                 TB[:, 2 * g + 1])

        # column interp per channel
        O = small_pool.tile([128, G, CROP * C], f32, tag=f"O{s}")
        for c in range(C):
            rts = []
            for k in range(K):
                xr = min(128, S - 128 * k)
                pst = pst_pool.tile([128, 256], f32, tag="pst")
                for g in range(G):
                    src = TB[:, 2 * g].rearrange(
                        "p (x c) -> p x c", c=C)[:, 128 * k:128 * k + xr, c]
                    nc.tensor.transpose(
                        out=pst[:xr, 128 * g:128 * (g + 1)],
                        in_=src,
                        identity=ident[:])
                rt = rt_pool.tile([128, CROP], f32, tag="rt")
                if k % 2 == 0:
                    nc.scalar.copy(rt[:xr], pst[:xr, :CROP])
                else:
                    nc.vector.tensor_copy(out=rt[:xr], in_=pst[:xr, :CROP])
                rts.append((rt, xr))
            pso = []
            for g in range(G):
                pso_t = pso_pool.tile([128, CROP], f32, tag="pso")
                pso.append(pso_t)
            for k in range(K):
                if p["jranges"][k] is None and k > 0:
                    continue
                if k == 0:
                    # full range: start=True must initialize every element
                    # later accumulating matmuls touch (wx cols outside
                    # this chunk's j-range are zero, so values are right)
                    jl, jh = 0, CROP
                else:
                    jl, jh = p["jranges"][k]
                rt, xr = rts[k]
                for g in range(G):
                    ng = 128 if g == 0 else CROP - 128
                    nc.tensor.matmul(
                        out=pso[g][:ng, jl:jh],
                        lhsT=rt[:xr, 128 * g:128 * g + ng],
                        rhs=wx_sb[:xr, k, jl:jh],
                        start=(k == 0),
                        stop=(k == K - 1),
                        skip_group_check=True,
                    )
            for g in range(G):
                ng = 128 if g == 0 else CROP - 128
                ov = O[:ng, g].rearrange("p (j c) -> p j c", c=C)[:, :, c]
                if g == 0:
                    nc.vector.tensor_copy(out=ov, in_=pso[g][:ng])
                else:
                    nc.scalar.copy(ov, pso[g][:ng])

        nc.sync.dma_start(out=outp[s, 0:128], in_=O[:, 0])
        nc.sync.dma_start(out=outp[s, 128:CROP], in_=O[0:CROP - 128, 1])


# ----------------------------------------------------------------------------
# Entry point
# ----------------------------------------------------------------------------

def _kernel_numpy_fallback(x, boxes, crop):
    b_idx = np.arange(x.shape[0])
    grid = np.arange(crop, dtype=np.float32) / np.float32(crop - 1)
    y1, x1, y2, x2 = boxes[:, 0], boxes[:, 1], boxes[:, 2], boxes[:, 3]
    hh, ww = x.shape[1], x.shape[2]
    in_y = (y1[:, None] + grid[None, :] * (y2 - y1)[:, None]) * np.float32(hh - 1)
    in_x = (x1[:, None] + grid[None, :] * (x2 - x1)[:, None]) * np.float32(ww - 1)
    valid_y = (in_y >= 0) & (in_y <= hh - 1)
    valid_x = (in_x >= 0) & (in_x <= ww - 1)
    top_f = np.floor(in_y)
    left_f = np.floor(in_x)
    yl = (in_y - top_f)[:, :, None, None].astype(np.float32)
    xl = (in_x - left_f)[:, None, :, None].astype(np.float32)
    t = np.clip(top_f.astype(np.int32), 0, hh - 1)
    b = np.clip(t + 1, 0, hh - 1)
    l = np.clip(left_f.astype(np.int32), 0, ww - 1)
    r = np.clip(l + 1, 0, ww - 1)
    bi = b_idx[:, None, None]
    tl = x[bi, t[:, :, None], l[:, None, :]]
    tr = x[bi, t[:, :, None], r[:, None, :]]
    bl = x[bi, b[:, :, None], l[:, None, :]]
    br = x[bi, b[:, :, None], r[:, None, :]]
    top_i = tl + (tr - tl) * xl
    bot_i = bl + (br - bl) * xl
    out = top_i + (bot_i - top_i) * yl
    valid = (valid_y[:, :, None] & valid_x[:, None, :])[..., None]
    return np.where(valid, out, np.float32(0.0)).astype(np.float32)


def _run(x, boxes, trace=False, trace_cores=None):
    from concourse.bass_utils import run_bass_kernel_spmd

    plans, assignment, kmax = _make_plans(boxes)
    in_maps = _build_host_inputs(x, boxes, plans, assignment, kmax)
    nc = _build_program(plans, assignment, kmax)
    res = run_bass_kernel_spmd(nc, in_maps, list(range(NCORES)),
                               trace=trace, trace_cores=trace_cores)

    out = np.empty((B, CROP, CROP, C), dtype=np.float32)
    for c in range(NCORES):
        core_out = res.results[c]["out"]
        for s in range(SLOTS):
            out[assignment[c][s]] = core_out[s].reshape(CROP, CROP, C)
    return out, res


def kernel(x, boxes, out_im_res):
    x = np.asarray(x, dtype=np.float32)
    boxes = np.asarray(boxes, dtype=np.float32)
    crop = int(out_im_res)
    if x.shape != (B, H, W, C) or crop != CROP:
        return _kernel_numpy_fallback(x, boxes, crop)
    return _run(x, boxes)[0]
